# revision 29
# baseline (speedup 1.0000x reference)
"""Causal attention (single-head, full hidden) on 8 TRN2 NeuronCores.

Reference (per batch element b, all fp32):
    q = x @ W_q.T + b_q ; k = x @ W_k.T + b_k ; v = x @ W_v.T + b_v
    s = q @ k.T / sqrt(H)   (causal masked)
    out = softmax(s) @ v

Sharding: pure data-parallel - batch B=8, one batch element per core.
Each core runs the identical graph on its own x slice; weights are
broadcast. No collectives.

Per-core structure (S=2048, D=H=1024), K^T resident from stage A on,
V resident from stage B on (no KV round trip / reload stall):
  Stage A (W_k + W_q resident): per 512-col s-range: DMA x, PE-transpose
          to xT, project KT[h,s] straight into resident KT_sb and
          QT[h,s] (bias folded, scaled 1/sqrt(H)) to per-range DRAM;
          spill xT to per-range DRAM.
  Stage B (W_v resident): per range: stream xT back, project V[s,h]
          straight into resident V_sb.
  Stage C: per 256-wide q-range: stream QT, causal flash attention with
          *transposed* scores ST[k,q] = KT.T @ QT, so exp(ST) is
          directly the lhsT of the P@V matmul (no P transposes).
          Softmax without max-subtraction (scores are O(10), exp safe
          in fp32); row sums via ones-vector matmul (fp32r needs even
          N, hence [P,2]); divide + V-bias fused in the epilogue.

Matmuls default to float32r (TF32-class: full PE rate at even free
dims, ~2e-4 rel err end to end); cdt="bf16" switches the
projection/attention operands to bfloat16 (~halves SBUF + DMA).
psum->SBUF copies alternate DVE/ACT so neither engine gates the PE.
DMA issue is spread across the SP HWDGE ring (nc.sync), ACT HWDGE ring
(nc.scalar) and SWDGE (nc.gpsimd).
"""

import sys

sys.path.insert(0, "/opt/trn_rl_repo")

import numpy as np

B, S, D, H = 8, 2048, 1024, 1024
P = 128  # partitions


def build(nc, tc, S=S, D=D, H=H, unroll=1, phases=(1, 2), cdt="f32r"):
    import concourse.bass as bass
    import concourse.mybir as mybir
    from concourse.masks import make_identity, make_upper_triangular
    from contextlib import ExitStack

    F32 = mybir.dt.float32
    F32R = mybir.dt.float32r              # dram inputs + transpose path
    CDT = mybir.dt.float32r if cdt == "f32r" else mybir.dt.bfloat16
    AF = mybir.ActivationFunctionType
    ALU = mybir.AluOpType

    x = nc.dram_tensor("x", [S, D], F32R, kind="ExternalInput")
    W_q = nc.dram_tensor("W_q", [H, D], F32R, kind="ExternalInput")
    b_q = nc.dram_tensor("b_q", [H], F32, kind="ExternalInput")
    W_k = nc.dram_tensor("W_k", [H, D], F32R, kind="ExternalInput")
    b_k = nc.dram_tensor("b_k", [H], F32, kind="ExternalInput")
    W_v = nc.dram_tensor("W_v", [H, D], F32R, kind="ExternalInput")
    b_v = nc.dram_tensor("b_v", [H], F32, kind="ExternalInput")
    out = nc.dram_tensor("out", [S, H], F32, kind="ExternalOutput")

    n_dc = D // P           # d chunks
    n_hc = H // P           # h chunks
    n_st = S // P           # s tiles of 128
    SR = min(512, S)        # s-range for projections
    n_sr = S // SR
    n_ss = SR // P          # s subtiles per range
    QR = min(256, S)        # flash q-range
    n_qr = S // QR
    n_qt = QR // P          # q tiles per range
    HW = min(512, H)        # psum-bank-wide h split
    n_hh = H // HW
    scale = 1.0 / float(np.sqrt(H))

    for _u in range(unroll):
        with ExitStack() as outer:
            singles = outer.enter_context(tc.tile_pool(name="singles", bufs=1))
            dram = outer.enter_context(tc.tile_pool(name="dram", bufs=1,
                                                    space="DRAM"))
            kv = outer.enter_context(tc.tile_pool(name="kv", bufs=1))

            # --- constants ---
            identity_f = singles.tile([P, P], F32)
            make_identity(nc, identity_f[:])
            identity = singles.tile([P, P], F32R)
            nc.vector.tensor_copy(out=identity[:], in_=identity_f[:])
            trildt = F32 if cdt == "f32r" else CDT
            tril = singles.tile([P, P], trildt)  # tril[k, q] = 1 where k <= q
            make_upper_triangular(nc, tril[:], val=1.0, diag=True)
            ones_f = singles.tile([P, 2], F32)
            nc.vector.memset(ones_f[:], 1.0)
            ones = singles.tile([P, 2], CDT)
            nc.vector.tensor_copy(out=ones[:], in_=ones_f[:])

            # biases: bqs[p, c] = b_q[c*128 + p], pre-scaled by 1/sqrt(H)
            bqs = singles.tile([P, n_hc], F32)
            nc.sync.dma_start(out=bqs[:], in_=b_q[:].rearrange("(c p) -> p c", p=P))
            nc.vector.tensor_scalar_mul(bqs[:], bqs[:], float(scale))
            bks = singles.tile([P, n_hc], F32)
            nc.sync.dma_start(out=bks[:], in_=b_k[:].rearrange("(c p) -> p c", p=P))
            bvb = singles.tile([P, H], F32)  # b_v broadcast across partitions
            nc.sync.dma_start(
                out=bvb[:], in_=bass.AP(tensor=b_v, offset=0, ap=[[0, P], [1, H]]))

            # resident K^T (V's pool opens after stage A frees its space)
            KT = kv.tile([P, n_hc, S], CDT, name="kt", tag="kt")
            V = None

            # per-range DRAM scratch
            xT_d = [dram.tile([P, n_dc, SR], CDT, name=f"xtd{_u}_{r}")
                    for r in range(n_sr)]
            QT_d = [dram.tile([n_hc, P, SR], CDT, name=f"qtd{_u}_{r}")
                    for r in range(n_sr)]

            def load_wT(pool, wst_pool, W, name, tp_ps=None, dma=None):
                """DMA W [h,d], PE-transpose to [d_p, dc, h] resident tile."""
                WT = pool.tile([P, n_dc, H], CDT, name=name, tag=name)
                with ExitStack() as es:
                    if tp_ps is None:
                        tp_ps = es.enter_context(
                            tc.tile_pool(name=f"tp_{name}", bufs=4, space="PSUM"))
                    for hc in range(n_hc):
                        win = wst_pool.tile([P, D], F32R, name="win", tag="win")
                        (dma or nc.sync).dma_start(
                            out=win[:], in_=W[hc * P:(hc + 1) * P, :])
                        for dc in range(n_dc):
                            tp = tp_ps.tile([P, P], F32R)
                            nc.tensor.transpose(
                                tp[:], win[:, dc * P:(dc + 1) * P], identity[:])
                            if dc % 2 == 0:
                                nc.vector.tensor_copy(
                                    out=WT[:, dc, hc * P:(hc + 1) * P], in_=tp[:])
                            else:
                                nc.scalar.activation(
                                    out=WT[:, dc, hc * P:(hc + 1) * P],
                                    in_=tp[:], func=AF.Copy)
                return WT

            def stage_a():
                with ExitStack() as st_:
                    wkp = st_.enter_context(tc.tile_pool(name="wk", bufs=1))
                    wqp = st_.enter_context(tc.tile_pool(name="wq", bufs=1))
                    wst = st_.enter_context(tc.tile_pool(name="wst", bufs=3))
                    xpool = st_.enter_context(tc.tile_pool(name="xin", bufs=4))
                    xtp = st_.enter_context(tc.tile_pool(name="xta", bufs=1))
                    stg = st_.enter_context(tc.tile_pool(name="stg", bufs=4))
                    tp_ps = st_.enter_context(
                        tc.tile_pool(name="tp_ps", bufs=4, space="PSUM"))
                    pj_ps = st_.enter_context(
                        tc.tile_pool(name="pj_ps", bufs=2, space="PSUM"))
                    W_kT = load_wT(wkp, wst, W_k, "wk", tp_ps=tp_ps)
                    W_qT = load_wT(wqp, wst, W_q, "wq", tp_ps=tp_ps,
                                   dma=nc.scalar)
                    for r in range(n_sr):
                        rs = r * SR
                        xT = xtp.tile([P, n_dc, SR], CDT)
                        for ss in range(n_ss):
                            xin = xpool.tile([P, D], F32R)
                            nc.sync.dma_start(
                                out=xin[:],
                                in_=x[rs + ss * P:rs + (ss + 1) * P, :])
                            for dc in range(n_dc):
                                tp = tp_ps.tile([P, P], F32R)
                                nc.tensor.transpose(
                                    tp[:], xin[:, dc * P:(dc + 1) * P],
                                    identity[:])
                                if dc % 2 == 0:
                                    nc.vector.tensor_copy(
                                        out=xT[:, dc, ss * P:(ss + 1) * P],
                                        in_=tp[:])
                                else:
                                    nc.scalar.activation(
                                        out=xT[:, dc, ss * P:(ss + 1) * P],
                                        in_=tp[:], func=AF.Copy)
                        nc.sync.dma_start(out=xT_d[r][:], in_=xT[:])
                        for hc in range(n_hc):
                            ps = pj_ps.tile([P, SR], F32)
                            for dc in range(n_dc):
                                nc.tensor.matmul(
                                    ps[:], W_kT[:, dc, hc * P:(hc + 1) * P],
                                    xT[:, dc, :],
                                    start=(dc == 0), stop=(dc == n_dc - 1))
                            nc.scalar.activation(
                                out=KT[:, hc, rs:rs + SR], in_=ps[:],
                                func=AF.Identity, bias=bks[:, hc:hc + 1],
                                scale=1.0)
                            qs_ = pj_ps.tile([P, SR], F32, name="qps")
                            for dc in range(n_dc):
                                nc.tensor.matmul(
                                    qs_[:], W_qT[:, dc, hc * P:(hc + 1) * P],
                                    xT[:, dc, :],
                                    start=(dc == 0), stop=(dc == n_dc - 1))
                            st = stg.tile([P, SR], CDT, name="qstg", tag="qstg")
                            nc.scalar.activation(
                                out=st[:], in_=qs_[:], func=AF.Identity,
                                bias=bqs[:, hc:hc + 1], scale=float(scale))
                            nc.scalar.dma_start(out=QT_d[r][hc, :, :], in_=st[:])

            def stage_b(V):
                with ExitStack() as st_:
                    wvp = st_.enter_context(tc.tile_pool(name="wv", bufs=1))
                    wstb = st_.enter_context(tc.tile_pool(name="wstb", bufs=3))
                    xtp = st_.enter_context(tc.tile_pool(name="xtb", bufs=3))
                    W_vT = load_wT(wvp, wstb, W_v, "wv", dma=nc.scalar)
                    v_ps = st_.enter_context(
                        tc.tile_pool(name="v_ps", bufs=3, space="PSUM"))
                    for r in range(n_sr):
                        for ss in range(n_ss):
                            xc = xtp.tile([P, n_dc, P], CDT, name="xc")
                            nc.sync.dma_start(
                                out=xc[:],
                                in_=xT_d[r][:, :, ss * P:(ss + 1) * P])
                            vp = v_ps.tile([P, H], F32)
                            for dc in range(n_dc):
                                for hh in range(n_hh):
                                    nc.tensor.matmul(
                                        vp[:, hh * HW:(hh + 1) * HW],
                                        xc[:, dc, :],
                                        W_vT[:, dc, hh * HW:(hh + 1) * HW],
                                        start=(dc == 0), stop=(dc == n_dc - 1))
                            if ss % 2 == 0:
                                nc.vector.tensor_copy(
                                    out=V[:, r * n_ss + ss, :], in_=vp[:])
                            else:
                                nc.scalar.activation(
                                    out=V[:, r * n_ss + ss, :], in_=vp[:],
                                    func=AF.Copy)

            def stage_c(V):
                with ExitStack() as st_:
                    qt_pool = st_.enter_context(tc.tile_pool(name="qt", bufs=2))
                    est_pool = st_.enter_context(tc.tile_pool(name="est", bufs=4))
                    osb_pool = st_.enter_context(tc.tile_pool(name="osb", bufs=2))
                    lsb_pool = st_.enter_context(tc.tile_pool(name="lsb", bufs=4))
                    st_ps = st_.enter_context(
                        tc.tile_pool(name="st_ps", bufs=2, space="PSUM"))
                    o_ps = st_.enter_context(
                        tc.tile_pool(name="o_ps", bufs=1, space="PSUM"))
                    l_ps = st_.enter_context(
                        tc.tile_pool(name="l_ps", bufs=1, space="PSUM"))

                    for qr in range(n_qr):
                        qs = qr * QR
                        n_j = qr * n_qt + n_qt
                        r, ro = qs // SR, qs % SR
                        QT = qt_pool.tile([P, n_hc, QR], CDT)
                        nc.sync.dma_start(
                            out=QT[:],
                            in_=QT_d[r].rearrange("hc p s -> p hc s")[
                                :, :, ro:ro + QR])

                        o_tiles = [o_ps.tile([P, H], F32, name=f"o{t}",
                                             tag=f"o{t}") for t in range(n_qt)]
                        l_tiles = [l_ps.tile([P, 2], F32, name=f"l{t}",
                                             tag=f"l{t}") for t in range(n_qt)]

                        for j in range(n_j):
                            ST = st_ps.tile([P, QR], F32)
                            for hc in range(n_hc):
                                nc.tensor.matmul(
                                    ST[:], KT[:, hc, j * P:(j + 1) * P],
                                    QT[:, hc, :],
                                    start=(hc == 0), stop=(hc == n_hc - 1))
                            est = est_pool.tile([P, QR], CDT)
                            nc.scalar.activation(out=est[:], in_=ST[:],
                                                 func=AF.Exp)
                            for t in range(n_qt):
                                gt = qr * n_qt + t
                                if j > gt:
                                    continue
                                if j == gt:  # diagonal block
                                    nc.gpsimd.tensor_tensor(
                                        out=est[:, t * P:(t + 1) * P],
                                        in0=est[:, t * P:(t + 1) * P],
                                        in1=tril[:], op=ALU.mult)
                                last = j == gt
                                for hh in range(n_hh):
                                    nc.tensor.matmul(
                                        o_tiles[t][:, hh * HW:(hh + 1) * HW],
                                        est[:, t * P:(t + 1) * P],
                                        V[:, j, hh * HW:(hh + 1) * HW],
                                        start=(j == 0), stop=last)
                                nc.tensor.matmul(
                                    l_tiles[t][:, 0:2],
                                    est[:, t * P:(t + 1) * P], ones[:],
                                    start=(j == 0), stop=last)

                        osb = osb_pool.tile([P, n_qt, H], F32, name="osb")
                        for t in range(n_qt):
                            linv = lsb_pool.tile([P, 1], F32, name="linv")
                            nc.vector.reciprocal(out=linv[:],
                                                 in_=l_tiles[t][:, 0:1])
                            nc.vector.scalar_tensor_tensor(
                                out=osb[:, t, :], in0=o_tiles[t][:],
                                scalar=linv[:], in1=bvb[:],
                                op0=ALU.mult, op1=ALU.add)
                        nc.scalar.dma_start(
                            out=out[qs:qs + QR, :].rearrange(
                                "(t p) h -> p t h", p=P),
                            in_=osb[:])

            vstack = outer.enter_context(ExitStack())
            if 1 in phases:
                stage_a()
                vpool = vstack.enter_context(tc.tile_pool(name="vpool", bufs=1))
                V = vpool.tile([P, n_st, H], CDT, name="v", tag="v")
                stage_b(V)
            else:
                # phase-2-only: fill resident tiles/scratch so reads are legal
                vpool = vstack.enter_context(tc.tile_pool(name="vpool", bufs=1))
                V = vpool.tile([P, n_st, H], CDT, name="v", tag="v")
                filler = singles.tile([P, max(S, n_dc * SR)], CDT,
                                      name=f"fill{_u}")
                nc.vector.memset(filler[:].bitcast(F32), 0.0)
                for hc in range(n_hc):
                    nc.vector.tensor_copy(out=KT[:, hc, :], in_=filler[:, 0:S])
                for st in range(n_st):
                    nc.vector.tensor_copy(out=V[:, st, :], in_=filler[:, 0:H])
                for r in range(n_sr):
                    nc.sync.dma_start(out=xT_d[r][:],
                                      in_=filler[:, 0:n_dc * SR])
                    for hc in range(n_hc):
                        nc.sync.dma_start(out=QT_d[r][hc, :, :],
                                          in_=filler[:, 0:SR])
            if 2 in phases:
                stage_c(V)
            else:
                zo = singles.tile([P, H], F32, name=f"zo{_u}")
                nc.vector.tensor_copy(out=zo[:], in_=bvb[:])
                # consume KT/V so releases are clean
                nc.vector.tensor_tensor(out=zo[:, 0:1],
                                        in0=KT[:, 0, 0:1].bitcast(F32),
                                        in1=V[:, 0, 0:1].bitcast(F32),
                                        op=ALU.add)
                nc.sync.dma_start(out=out[0:P, :], in_=zo[:])


_CACHE = {}


def _get_compiled(S=S, D=D, H=H, unroll=1, phases=(1, 2), cdt="f32r"):
    key = (S, D, H, unroll, tuple(phases), cdt)
    if key not in _CACHE:
        import concourse.tile as tile
        from concourse import bacc

        nc = bacc.Bacc("TRN2", target_bir_lowering=False, debug=False)
        with tile.TileContext(nc) as tc:
            build(nc, tc, S=S, D=D, H=H, unroll=unroll, phases=phases, cdt=cdt)
        nc.compile()
        _CACHE[key] = nc
    return _CACHE[key]


def kernel(x, W_q, b_q, W_k, b_k, W_v, b_v):
    from concourse.bass_utils import run_bass_kernel_spmd

    nc = _get_compiled()
    x = np.ascontiguousarray(np.asarray(x, dtype=np.float32))
    common = {
        "W_q": np.ascontiguousarray(np.asarray(W_q, np.float32)),
        "b_q": np.ascontiguousarray(np.asarray(b_q, np.float32)),
        "W_k": np.ascontiguousarray(np.asarray(W_k, np.float32)),
        "b_k": np.ascontiguousarray(np.asarray(b_k, np.float32)),
        "W_v": np.ascontiguousarray(np.asarray(W_v, np.float32)),
        "b_v": np.ascontiguousarray(np.asarray(b_v, np.float32)),
    }
    in_maps = [{"x": x[i], **common} for i in range(B)]
    r = run_bass_kernel_spmd(nc, in_maps, core_ids=list(range(B)))
    return np.stack([r.results[i]["out"] for i in range(B)])
